# revision 24
# baseline (speedup 1.0000x reference)
"""Trainium2 Bass kernel for BEVHDMapFusionNet.

Data-parallel over B*T: 8 frames -> 8 NeuronCores, one frame per core.

Per-frame pipeline (all on one core):
  conv3x3(144->128) on [bev|ego]  -> bev_feat          (query source)
  conv3x3(64->128) on hd_map      -> hd_feat
  bilinear 2x upsample of front   -> front_rs
  kv = [hd_feat | front_rs]  (192 ch)
  Qt/Kt = w @ feat  ([head*dim, 1024] layouts), V = kv.T @ wv.T ([k,128])
  per (kc, qh): scoresT = Kt_h.T @ Qt_h  (4 heads row-tiled on the PE)
               P = exp(scale*scoresT)    (ScalarE, no max-subtraction: scores are O(1))
               attn += V_h.T @ P, den += 1.T @ P  (4 heads col-tiled)
  attnT = attn * recip(den); fused = woT.T @ attnT + bo
  conv3x3(144->128) on [fused|ego] -> out

Convs are 9 shifted matmuls over a zero-padded [C, 34, 34] SBUF image; the
ego (spatially-constant) channels + bias enter as a rank-10 matmul against
precomputed border-indicator maps.

All matmul operands are float32r (single-pass full-rate fp32 PE mode); the
verifier requires operands to be *rounded* by a compute op, so every matmul
input tile is written by a DVE/ACT instruction with a float32r output.
"""

import math
from itertools import product

import numpy as np

import concourse.bass as bass
import concourse.mybir as mybir
import concourse.tile as tile
from concourse.bacc import Bacc
from concourse.bass import ts
from concourse.bass_utils import run_bass_kernel_spmd
from concourse.masks import make_identity

F32 = mybir.dt.float32
B16 = mybir.dt.bfloat16
AF = mybir.ActivationFunctionType
OP = mybir.AluOpType

NUM_HEADS = 4
HEAD_DIM = 32
SCALE = 1.0 / math.sqrt(HEAD_DIM)

# Matmul-operand dtype: float32r = single-pass (full-rate) fp32 PE mode.
# Set to F32 for exact-but-4x-slower matmuls.
MMDT = mybir.dt.float32r

TAPS = list(product(range(3), range(3)))  # j = ky*3 + kx


def _emit_conv(nc, ps, x_pad, wT, nchan, extra_lhsT, extra_rhs):
    """3x3 SAME conv: accumulate 9 shifted matmuls + one extra (ego/bias) matmul.

    ps:    PSUM [128, 2, 512]
    x_pad: SBUF [nchan, 34, 34] zero-padded image (MMDT)
    wT:    SBUF [nchan, 9, 128] per-tap transposed weights (MMDT)
    extra_lhsT/extra_rhs: final accumulated matmul (ego taps + bias row)
    """
    for qh in range(2):
        for j, (ky, kx) in enumerate(TAPS):
            nc.tensor.matmul(
                ps[:, qh, :],
                wT[:, j, :],
                x_pad[:nchan, ky + 16 * qh : ky + 16 * qh + 16, kx : kx + 32],
                start=(j == 0),
                stop=False,
            )
        nc.tensor.matmul(
            ps[:, qh, :],
            extra_lhsT,
            extra_rhs[:, 16 * qh : 16 * qh + 16, :],
            start=False,
            stop=True,
        )


def _emit_resize(nc, work, front_sb, front_rs):
    """jax.image.resize bilinear 16->32 (align_corners=False), separable.

    out[0]=in[0]; out[31]=in[15]; out[2i]=.25 in[i-1]+.75 in[i];
    out[2i+1]=.75 in[i]+.25 in[i+1]
    """
    fx = work.tile([64, 16, 32], F32, tag="fx", bufs=1)
    # x axis
    nc.vector.tensor_copy(fx[:, :, 0], front_sb[:, :, 0])
    nc.vector.tensor_copy(fx[:, :, 31], front_sb[:, :, 15])
    fxv = fx.rearrange("p i (a b) -> p i a b", b=2)
    te = work.tile([64, 16, 15], F32, tag="te", bufs=2)
    nc.vector.tensor_scalar_mul(te, front_sb[:, :, 0:15], 1.0 / 3.0)
    nc.vector.tensor_add(te, te, front_sb[:, :, 1:16])
    nc.vector.tensor_scalar_mul(fxv[:, :, 1:16, 0], te, 0.75)
    to = work.tile([64, 16, 15], F32, tag="te", bufs=2)
    nc.vector.tensor_scalar_mul(to, front_sb[:, :, 0:15], 3.0)
    nc.vector.tensor_add(to, to, front_sb[:, :, 1:16])
    nc.vector.tensor_scalar_mul(fxv[:, :, 0:15, 1], to, 0.25)
    # y axis (writes MMDT front_rs)
    nc.vector.tensor_copy(front_rs[:, 0, :], fx[:, 0, :])
    nc.vector.tensor_copy(front_rs[:, 31, :], fx[:, 15, :])
    fyv = front_rs.rearrange("p (a b) x -> p a b x", b=2)
    ye = work.tile([64, 15, 32], F32, tag="ty", bufs=2)
    nc.vector.tensor_scalar_mul(ye, fx[:, 0:15, :], 1.0 / 3.0)
    nc.vector.tensor_add(ye, ye, fx[:, 1:16, :])
    nc.vector.tensor_scalar_mul(fyv[:, 1:16, 0, :], ye, 0.75)
    yo = work.tile([64, 15, 32], F32, tag="ty", bufs=2)
    nc.vector.tensor_scalar_mul(yo, fx[:, 0:15, :], 3.0)
    nc.vector.tensor_add(yo, yo, fx[:, 1:16, :])
    nc.vector.tensor_scalar_mul(fyv[:, 0:15, 1, :], yo, 0.25)


def build_module(debug_taps=False):
    # Bacc (not plain Bass): its finalize() runs the wait-splitting compile
    # passes (generate_event_semaphores etc.) the TRN2 ISA requires — each
    # instruction can carry at most one semaphore wait.
    nc = Bacc()
    dbg = {}
    if debug_taps:
        for nm, shp in [
            ("d_bev_feat", [128, 1024]), ("d_hd_feat", [128, 1024]),
            ("d_front", [64, 1024]), ("d_Qt", [128, 1024]), ("d_Kt", [128, 1024]),
            ("d_V", [128, 1024]), ("d_attn", [128, 1024]), ("d_den", [128, 1024]),
            ("d_attnT", [128, 1024]), ("d_fused", [128, 1156]),
            ("d_a10", [10, 128]), ("d_ones10", [10, 1024]), ("d_ebc", [128, 16]),
        ]:
            dbg[nm] = nc.dram_tensor(nm, shp, F32, kind="ExternalOutput")

    # ---- DRAM I/O (per-core frame slice + shared weights) ----
    bev = nc.dram_tensor("bev", [128, 32, 32], F32, kind="ExternalInput")
    hd = nc.dram_tensor("hd", [64, 32, 32], F32, kind="ExternalInput")
    ego = nc.dram_tensor("ego", [1, 16], F32, kind="ExternalInput")
    front = nc.dram_tensor("front", [64, 16, 16], F32, kind="ExternalInput")
    w_bev = nc.dram_tensor("w_bev", [128, 1296], F32, kind="ExternalInput")
    b_bev = nc.dram_tensor("b_bev", [128, 1], F32, kind="ExternalInput")
    w_hd = nc.dram_tensor("w_hd", [128, 576], F32, kind="ExternalInput")
    b_hd = nc.dram_tensor("b_hd", [1, 128], F32, kind="ExternalInput")
    wq = nc.dram_tensor("wq", [128, 128], F32, kind="ExternalInput")
    wk = nc.dram_tensor("wk", [128, 192], F32, kind="ExternalInput")
    wv = nc.dram_tensor("wv", [128, 192], F32, kind="ExternalInput")
    wo = nc.dram_tensor("wo", [128, 128], F32, kind="ExternalInput")
    bo = nc.dram_tensor("bo", [128, 1], F32, kind="ExternalInput")
    w_out = nc.dram_tensor("w_out", [128, 1296], F32, kind="ExternalInput")
    b_out = nc.dram_tensor("b_out", [128, 1], F32, kind="ExternalInput")
    out = nc.dram_tensor("out", [128, 1024], F32, kind="ExternalOutput")

    with tile.TileContext(nc) as tc:
        with (
            tc.tile_pool(name="persist", bufs=1) as pp,
            tc.tile_pool(name="work", bufs=2) as work,
            tc.tile_pool(name="pP", bufs=2) as pP,
            tc.tile_pool(name="psA", bufs=1, space=bass.MemorySpace.PSUM) as psA,
            tc.tile_pool(name="psS", bufs=2, space=bass.MemorySpace.PSUM) as psS,
        ):
            # ---------- loads + fp32r rounding ----------
            bev_pad = pp.tile([128, 34, 34], MMDT)
            hd_pad = pp.tile([64, 34, 34], MMDT)
            fused_pad = pp.tile([128, 34, 34], MMDT)

            # Zero only the 1-px borders of the padded fp32r images: the
            # interior writers then have no same-engine WAW hazard, keeping
            # every fp32r-writing instruction at <=1 sync wait (the fp32r
            # rounding datapath instruction format only has one wait slot).
            zeros_f = pp.tile([128, 34, 34], F32)
            nc.gpsimd.memset(zeros_f[:, :, :], 0.0)
            for pad, np_ in ((bev_pad, 128), (hd_pad, 64), (fused_pad, 128)):
                nc.vector.tensor_copy(pad[:, 0:1, :], zeros_f[:np_, 0:1, :])
                nc.vector.tensor_copy(pad[:, 33:34, :], zeros_f[:np_, 33:34, :])
                nc.vector.tensor_copy(pad[:, 1:33, 0:1], zeros_f[:np_, 1:33, 0:1])
                nc.vector.tensor_copy(pad[:, 1:33, 33:34], zeros_f[:np_, 1:33, 33:34])

            bev_ld = work.tile([128, 32, 32], F32, tag="bev_ld", bufs=1)
            nc.sync.dma_start(bev_ld[:, :, :], bev[:, :, :])
            nc.vector.tensor_copy(bev_pad[:, 1:33, 1:33], bev_ld[:, :, :])

            hd_ld = work.tile([64, 32, 32], F32, tag="hd_ld", bufs=1)
            nc.sync.dma_start(hd_ld[:, :, :], hd[:, :, :])
            nc.vector.tensor_copy(hd_pad[:, 1:33, 1:33], hd_ld[:, :, :])

            front_sb = pp.tile([64, 16, 16], F32)
            nc.sync.dma_start(front_sb[:, :, :], front[:, :, :])

            w_bev_sb = pp.tile([128, 1296], F32)
            w_hd_sb = pp.tile([128, 576], F32)
            w_out_sb = pp.tile([128, 1296], F32)
            nc.sync.dma_start(w_bev_sb[:, :], w_bev[:, :])
            nc.sync.dma_start(w_hd_sb[:, :], w_hd[:, :])
            nc.sync.dma_start(w_out_sb[:, :], w_out[:, :])

            wq_sb = pp.tile([128, 128], F32)
            wk_sb = pp.tile([128, 192], F32)
            wv_sb = pp.tile([128, 192], F32)
            wo_sb = pp.tile([128, 128], F32)
            nc.sync.dma_start(wq_sb[:, :], wq[:, :])
            nc.sync.dma_start(wk_sb[:, :], wk[:, :])
            nc.sync.dma_start(wv_sb[:, :], wv[:, :])
            nc.sync.dma_start(wo_sb[:, :], wo[:, :])

            bo_sb = pp.tile([128, 1], F32)
            nc.sync.dma_start(bo_sb[:, :], bo[:, :])
            bhd_f = work.tile([1, 128], F32, tag="brow", bufs=2)
            nc.sync.dma_start(bhd_f[:, :], b_hd[:, :])
            bhd_sb = pp.tile([1, 128], MMDT)
            nc.vector.tensor_copy(bhd_sb[:, :], bhd_f[:, :])

            # ego broadcast across partitions: e_bc[p, c] = ego[c]
            e_bc = pp.tile([128, 16], F32)
            nc.sync.dma_start(e_bc[:, :], ego[:, :].to_broadcast([128, 16]))

            # ---------- constants ----------
            ident = pp.tile([128, 128], F32)
            make_identity(nc, ident[:, :])

            # ones10[j] = tap-j validity map over output pixels; row 9 = all-ones.
            # Compute-engine writes must start at partition 0/32/64/96, so the
            # 10 rows are staged in partition 0 and DMA-scattered to partitions,
            # then rounded to fp32r by a DVE copy.
            ones_stage = work.tile([1, 10, 32, 32], F32, tag="ones_stage", bufs=1)
            nc.gpsimd.memset(ones_stage[:, :, :, :], 0.0)
            for j, (ky, kx) in enumerate(TAPS):
                y0, y1 = (1, 32) if ky == 0 else (0, 31) if ky == 2 else (0, 32)
                x0, x1 = (1, 32) if kx == 0 else (0, 31) if kx == 2 else (0, 32)
                nc.gpsimd.memset(ones_stage[0:1, j, y0:y1, x0:x1], 1.0)
            nc.gpsimd.memset(ones_stage[0:1, 9, :, :], 1.0)
            ones10_f = work.tile([10, 32, 32], F32, tag="ones10_f", bufs=1)
            nc.sync.dma_start(ones10_f[:, :, :], ones_stage[0:1, :, :, :])
            ones10 = pp.tile([10, 32, 32], MMDT)
            nc.vector.tensor_copy(ones10[:, :, :], ones10_f[:, :, :])
            ones1 = pp.tile([1, 32, 32], MMDT)
            nc.vector.tensor_copy(ones1[:, :, :], ones_stage[0:1, 9, :, :])

            # bf16 all-ones stationary for the softmax-denominator matmuls
            ones32 = pp.tile([128, 32], B16)
            nc.gpsimd.memset(ones32[:, :], 1.0)

            # ---------- weight transposes (PE), copies round to fp32r ----------
            def pe_transpose(dst, src):
                tp = psS.tile([128, 2, 512], F32, tag="sc")
                tview = tp.rearrange("p a b -> p (a b)")
                nparts = src.shape[-1]  # out partitions = src free size
                nc.tensor.transpose(tview[:nparts, 0:128], src, ident[:, :])
                nc.vector.tensor_copy(dst, tview[:nparts, 0:128])

            wqT = pp.tile([128, 128], MMDT)
            pe_transpose(wqT[:, :], wq_sb[:, :])
            woT = pp.tile([128, 128], MMDT)
            pe_transpose(woT[:, :], wo_sb[:, :])
            wkT_a = pp.tile([128, 128], MMDT)
            pe_transpose(wkT_a[:, :], wk_sb[:, 0:128])
            wkT_b = pp.tile([64, 128], MMDT)
            pe_transpose(wkT_b[:, :], wk_sb[:, 128:192])
            wvT_a = pp.tile([128, 128], MMDT)
            pe_transpose(wvT_a[:, :], wv_sb[:, 0:128])
            wvT_b = pp.tile([64, 128], MMDT)
            pe_transpose(wvT_b[:, :], wv_sb[:, 128:192])

            w_bevT = pp.tile([128, 9, 128], MMDT)
            w_hdT = pp.tile([64, 9, 128], MMDT)
            w_outT = pp.tile([128, 9, 128], MMDT)
            wbv = w_bev_sb.rearrange("p (c j) -> p c j", j=9)
            whv = w_hd_sb.rearrange("p (c j) -> p c j", j=9)
            wov = w_out_sb.rearrange("p (c j) -> p c j", j=9)
            for j in range(9):
                pe_transpose(w_bevT[:, j, :], wbv[:, 0:128, j])
                pe_transpose(w_hdT[:, j, :], whv[:, 0:64, j])
                pe_transpose(w_outT[:, j, :], wov[:, 0:128, j])

            # ---------- ego tap-sum matrices A10 = [A[j,o] rows; bias row] ----------
            def build_a10(w_sb, b_col, label):
                wev = w_sb.rearrange("p (c j) -> p c j", j=9)  # c in [0,144)
                a_t = work.tile([128, 10], F32, tag="a_t", bufs=2)
                for j in range(9):
                    prd = work.tile([128, 16], F32, tag="prd", bufs=2)
                    nc.vector.tensor_mul(prd, wev[:, 128:144, j], e_bc[:, :])
                    nc.vector.tensor_reduce(
                        a_t[:, j : j + 1], prd, axis=mybir.AxisListType.X, op=OP.add
                    )
                nc.sync.dma_start(a_t[:, 9:10], b_col[:, :])
                a10 = pp.tile([10, 128], MMDT, name=f"a10_{label}")
                tp = psS.tile([128, 2, 512], F32, tag="sc")
                tview = tp.rearrange("p a b -> p (a b)")
                nc.tensor.transpose(tview[:10, 0:128], a_t[:, :], ident[:, :])
                nc.vector.tensor_copy(a10[:, :], tview[:10, 0:128])
                return a10

            a10_bev = build_a10(w_bev_sb, b_bev, "bev")
            a10_out = build_a10(w_out_sb, b_out, "out")

            # ---------- front resize ----------
            front_rs = pp.tile([64, 32, 32], MMDT)
            _emit_resize(nc, work, front_sb, front_rs)
            front_flat = front_rs.rearrange("p a b -> p (a b)")

            # ---------- convs ----------
            bev_feat = pp.tile([128, 1024], MMDT)
            cps = psA.tile([128, 2, 512], F32, tag="accA")
            _emit_conv(nc, cps, bev_pad, w_bevT, 128, a10_bev[:, :], ones10)
            nc.vector.tensor_scalar_max(
                bev_feat[:, :], cps.rearrange("p a b -> p (a b)"), 0.0
            )

            hd_feat = pp.tile([128, 1024], MMDT)
            hps = psA.tile([128, 2, 512], F32, tag="accB")
            _emit_conv(nc, hps, hd_pad, w_hdT, 64, bhd_sb[:, :], ones1)
            nc.vector.tensor_scalar_max(
                hd_feat[:, :], hps.rearrange("p a b -> p (a b)"), 0.0
            )

            # ---------- Q/K/V projections ----------
            Qt = pp.tile([128, 1024], MMDT)
            qps = psA.tile([128, 2, 512], F32, tag="accA")
            for qh in range(2):
                nc.tensor.matmul(qps[:, qh, :], wqT[:, :], bev_feat[:, ts(qh, 512)])
            nc.vector.tensor_copy(Qt[:, :], qps.rearrange("p a b -> p (a b)"))

            Kt = pp.tile([128, 1024], MMDT)
            kps = psA.tile([128, 2, 512], F32, tag="accB")
            for qh in range(2):
                nc.tensor.matmul(
                    kps[:, qh, :],
                    wkT_a[:, :],
                    hd_feat[:, ts(qh, 512)],
                    start=True,
                    stop=False,
                )
                nc.tensor.matmul(
                    kps[:, qh, :],
                    wkT_b[:, :],
                    front_flat[:, ts(qh, 512)],
                    start=False,
                    stop=True,
                )
            nc.vector.tensor_copy(Kt[:, :], kps.rearrange("p a b -> p (a b)"))

            V = pp.tile([128, 8, 128], B16)
            for kc in range(8):
                vps = psS.tile([128, 2, 512], F32, tag="sc")
                nc.tensor.matmul(
                    vps[:, 0, 0:128],
                    hd_feat[:, ts(kc, 128)],
                    wvT_a[:, :],
                    start=True,
                    stop=False,
                )
                nc.tensor.matmul(
                    vps[:, 0, 0:128],
                    front_flat[:, ts(kc, 128)],
                    wvT_b[:, :],
                    start=False,
                    stop=True,
                )
                nc.vector.tensor_copy(V[:, kc, :], vps[:, 0, 0:128])

            # ---------- attention ----------
            attn_ps = psA.tile([128, 2, 512], F32, tag="accA")
            den_ps = psA.tile([128, 2, 512], F32, tag="accB")
            for kc in range(8):
                Pk = pP.tile([128, 4, 1024], B16, tag="P")
                for h in range(4):
                    sc = psS.tile([128, 2, 512], F32, tag="sc")
                    for qh in range(2):
                        nc.tensor.matmul(
                            sc[:, qh, :],
                            Kt[32 * h : 32 * h + 32, ts(kc, 128)],
                            Qt[32 * h : 32 * h + 32, ts(qh, 512)],
                            tile_position=(32 * h, 0),
                        )
                    nc.scalar.activation(
                        Pk[:, h, :],
                        sc.rearrange("p a b -> p (a b)"),
                        AF.Exp,
                        scale=SCALE,
                    )
                for qh in range(2):
                    for h in range(4):
                        nc.tensor.matmul(
                            attn_ps[32 * h : 32 * h + 32, qh, :],
                            V[:, kc, 32 * h : 32 * h + 32],
                            Pk[:, h, ts(qh, 512)],
                            start=(kc == 0),
                            stop=(kc == 7),
                            tile_position=(0, 32 * h),
                        )
                    for h in range(4):
                        nc.tensor.matmul(
                            den_ps[32 * h : 32 * h + 32, qh, :],
                            ones32[:, :],
                            Pk[:, h, ts(qh, 512)],
                            start=(kc == 0),
                            stop=(kc == 7),
                            tile_position=(0, 32 * h),
                        )

            if debug_taps:
                nc.sync.dma_start(dbg["d_a10"][:, :], a10_bev[:, :].bitcast(F32))
                nc.sync.dma_start(
                    dbg["d_ones10"][:, :],
                    ones10.rearrange("p a b -> p (a b)").bitcast(F32),
                )
                nc.sync.dma_start(dbg["d_ebc"][:, :], e_bc[:, :])
                nc.sync.dma_start(dbg["d_bev_feat"][:, :], bev_feat[:, :].bitcast(F32))
                nc.sync.dma_start(dbg["d_hd_feat"][:, :], hd_feat[:, :].bitcast(F32))
                nc.sync.dma_start(dbg["d_front"][:, :], front_flat[:, :].bitcast(F32))
                nc.sync.dma_start(dbg["d_Qt"][:, :], Qt[:, :].bitcast(F32))
                nc.sync.dma_start(dbg["d_Kt"][:, :], Kt[:, :].bitcast(F32))
                vf = pp.tile([128, 1024], F32)
                nc.vector.tensor_copy(vf[:, :], V.rearrange("p a b -> p (a b)"))
                nc.sync.dma_start(dbg["d_V"][:, :], vf[:, :])
                af = pp.tile([128, 1024], F32)
                nc.vector.tensor_copy(af[:, :], attn_ps.rearrange("p a b -> p (a b)"))
                nc.sync.dma_start(dbg["d_attn"][:, :], af[:, :])
                df = pp.tile([128, 1024], F32)
                nc.vector.tensor_copy(df[:, :], den_ps.rearrange("p a b -> p (a b)"))
                nc.sync.dma_start(dbg["d_den"][:, :], df[:, :])

            recipB = pp.tile([128, 1024], F32)
            nc.vector.reciprocal(recipB[:, :], den_ps.rearrange("p a b -> p (a b)"))
            attnT = pp.tile([128, 1024], MMDT)
            nc.vector.tensor_mul(
                attnT[:, :], attn_ps.rearrange("p a b -> p (a b)"), recipB[:, :]
            )

            # ---------- output projection + out conv ----------
            fps = psA.tile([128, 2, 512], F32, tag="accA")
            for qh in range(2):
                nc.tensor.matmul(fps[:, qh, :], woT[:, :], attnT[:, ts(qh, 512)])
                nc.vector.tensor_scalar_add(
                    fused_pad[:, 1 + 16 * qh : 17 + 16 * qh, 1:33],
                    fps[:, qh, :].rearrange("p (a b) -> p a b", b=32),
                    bo_sb[:, :],
                )

            if debug_taps:
                nc.sync.dma_start(dbg["d_attnT"][:, :], attnT[:, :].bitcast(F32))
                nc.sync.dma_start(
                    dbg["d_fused"][:, :],
                    fused_pad.rearrange("p a b -> p (a b)").bitcast(F32),
                )

            out_sb = pp.tile([128, 1024], F32)
            ops_ = psA.tile([128, 2, 512], F32, tag="accB")
            _emit_conv(nc, ops_, fused_pad, w_outT, 128, a10_out[:, :], ones10)
            nc.vector.tensor_scalar_max(
                out_sb[:, :], ops_.rearrange("p a b -> p (a b)"), 0.0
            )
            nc.sync.dma_start(out[:, :], out_sb[:, :])

    nc.finalize()
    return nc


_NC = None
last_results = None


def kernel(**inputs) -> np.ndarray:
    global _NC, last_results
    import os

    if _NC is None:
        _NC = build_module(
            debug_taps=bool(int(os.environ.get("KERNEL_DEBUG_TAPS", "0")))
        )

    bev = np.ascontiguousarray(np.asarray(inputs["bev"], dtype=np.float32))
    hd_map = np.ascontiguousarray(np.asarray(inputs["hd_map"], dtype=np.float32))
    ego = np.ascontiguousarray(np.asarray(inputs["ego_info"], dtype=np.float32))
    front = np.ascontiguousarray(
        np.asarray(inputs["front_view_feature"], dtype=np.float32)
    )
    B, T = bev.shape[0], bev.shape[1]
    shared = {
        "w_bev": np.ascontiguousarray(
            np.asarray(inputs["w_bev"], np.float32).reshape(128, 1296)
        ),
        "b_bev": np.asarray(inputs["b_bev"], np.float32).reshape(1, 128).copy(),
        "w_hd": np.ascontiguousarray(
            np.asarray(inputs["w_hd"], np.float32).reshape(128, 576)
        ),
        "b_hd": np.asarray(inputs["b_hd"], np.float32).reshape(1, 128).copy(),
        "wq": np.ascontiguousarray(np.asarray(inputs["wq"], np.float32)),
        "wk": np.ascontiguousarray(np.asarray(inputs["wk"], np.float32)),
        "wv": np.ascontiguousarray(np.asarray(inputs["wv"], np.float32)),
        "wo": np.ascontiguousarray(np.asarray(inputs["wo"], np.float32)),
        "bo": np.asarray(inputs["bo"], np.float32).reshape(128, 1).copy(),
        "w_out": np.ascontiguousarray(
            np.asarray(inputs["w_out"], np.float32).reshape(128, 1296)
        ),
        "b_out": np.asarray(inputs["b_out"], np.float32).reshape(1, 128).copy(),
    }
    in_maps = []
    for i in range(8):
        b, t = divmod(i, T)
        m = dict(shared)
        m["bev"] = np.ascontiguousarray(bev[b, t])
        m["hd"] = np.ascontiguousarray(hd_map[b, t])
        m["ego"] = np.ascontiguousarray(ego[b, t].reshape(1, 16))
        m["front"] = np.ascontiguousarray(front[b, t])
        in_maps.append(m)

    res = run_bass_kernel_spmd(
        _NC,
        in_maps,
        core_ids=list(range(8)),
        trace=bool(int(os.environ.get("KERNEL_TRACE", "0"))),
    )
    last_results = res
    outs = np.stack([res.results[i]["out"] for i in range(8)])  # [8, 128, 1024]
    return outs.reshape(B, T, 128, 32, 32)
